# Initial kernel scaffold
#
"""Trainium2 Bass kernel for nn_ClusterClassifier (moe_routing).

Strategy: expert-parallel with host-side token routing.
  - Host groups tokens by cluster label; core i gets cluster i's tokens
    (gathered + transposed to [H, T]) and cluster i's decoder weight
    (pre-transposed to [H, vocab_i], LN gamma folded in, zero-padded).
  - Device (per core): h = xT.T @ W_t^T  -> erf-GELU -> LayerNorm
    (bn_stats + Newton-refined rsqrt) -> PE transpose -> logits =
    hn @ Wd'^T (+ folded bias) -> DMA out [T_cap, V_cap] fp32.
  - Host scatters the compact per-cluster logits into zero-filled full
    outputs (the mask semantics of the reference).

All matmuls run in float32r (TF32-class fp32 fast path, ~1e-4 rel err).
"""

import numpy as np

H = 768
KT = H // 128  # 6 contraction chunks
N_CLUSTERS = 8
VOCABS = [3000, 4000, 5000, 6000, 3000, 4000, 5000, 6000]
LN_EPS = 1e-12
VTILE = 512

_prog_cache: dict = {}


def _build_program(T_cap: int, V_cap: int, with_bt: bool, with_bd: bool):
    import concourse.bacc as bacc
    import concourse.mybir as mybir
    import concourse.tile as tile

    f32 = mybir.dt.float32
    f32r = mybir.dt.float32r
    AF = mybir.ActivationFunctionType
    ALU = mybir.AluOpType

    NT = T_cap // 128
    NV = V_cap // VTILE

    nc = bacc.Bacc("TRN2", target_bir_lowering=False, debug=False, num_devices=8)

    xT_d = nc.dram_tensor("xT", [H, T_cap], f32, kind="ExternalInput").ap()
    wtT_d = nc.dram_tensor("wtT", [H, H], f32, kind="ExternalInput").ap()
    wdT_d = nc.dram_tensor("wdT", [H, V_cap], f32, kind="ExternalInput").ap()
    id_d = nc.dram_tensor("ident", [128, 128], f32, kind="ExternalInput").ap()
    ones_d = nc.dram_tensor("ones1", [1, 128], f32, kind="ExternalInput").ap()
    if with_bt:
        bt_d = nc.dram_tensor("btT", [1, H], f32, kind="ExternalInput").ap()
    if with_bd:
        bd_d = nc.dram_tensor("bdT", [1, V_cap], f32, kind="ExternalInput").ap()
    out_d = nc.dram_tensor("out", [T_cap, V_cap], f32, kind="ExternalOutput").ap()

    out_tiled = out_d.rearrange("(nt p) (nv n) -> nt p nv n", p=128, n=VTILE)

    with tile.TileContext(nc) as tc:
        with (
            tc.tile_pool(name="consts", bufs=1) as consts,
            tc.tile_pool(name="hnt", bufs=1) as hnt_pool,
            tc.tile_pool(name="wd", bufs=3) as wd_pool,
            tc.tile_pool(name="work", bufs=2) as work,
            tc.tile_pool(name="stats", bufs=4) as stats,
            tc.tile_pool(name="outp", bufs=6) as outp,
            tc.tile_pool(name="hps", bufs=1, space="PSUM") as hps_pool,
            tc.tile_pool(name="tpps", bufs=2, space="PSUM") as tpps_pool,
            tc.tile_pool(name="decps", bufs=4, space="PSUM") as decps_pool,
        ):
            # ---- constants / full-kernel-lifetime tensors ----
            xT_sb = consts.tile([128, KT, T_cap], f32r)
            nc.gpsimd.dma_start(out=xT_sb, in_=xT_d.rearrange("(c p) t -> p c t", p=128))
            wtT_sb = consts.tile([128, KT, H], f32r)
            nc.gpsimd.dma_start(out=wtT_sb, in_=wtT_d.rearrange("(c p) o -> p c o", p=128))
            ident_sb = consts.tile([128, 128], f32r)
            nc.gpsimd.dma_start(out=ident_sb, in_=id_d)
            ones_sb = consts.tile([1, 128], f32r)
            nc.gpsimd.dma_start(out=ones_sb, in_=ones_d)
            eps_sb = consts.tile([128, 1], f32)
            nc.vector.memset(eps_sb, LN_EPS)
            if with_bt:
                btT_sb = consts.tile([1, H], f32r)
                nc.gpsimd.dma_start(out=btT_sb, in_=bt_d)
            if with_bd:
                bdT_sb = consts.tile([1, V_cap], f32r)
                nc.gpsimd.dma_start(out=bdT_sb, in_=bd_d)

            hnT_sb = hnt_pool.tile([128, KT, T_cap], f32r)

            # ---- phase 1: transform + layernorm + transpose ----
            for t in range(NT):
                h_ps = hps_pool.tile([128, H], f32)
                for lo, hi in ((0, 512), (512, 768)):
                    for k in range(KT):
                        nc.tensor.matmul(
                            h_ps[:, lo:hi],
                            xT_sb[:, k, t * 128:(t + 1) * 128],
                            wtT_sb[:, k, lo:hi],
                            start=(k == 0),
                            stop=(k == KT - 1 and not with_bt),
                        )
                    if with_bt:
                        nc.tensor.matmul(
                            h_ps[:, lo:hi], ones_sb, btT_sb[0:1, lo:hi],
                            start=False, stop=True,
                        )
                # GELU (erf variant) PSUM -> SBUF
                hg = work.tile([128, H], f32, tag="hg")
                nc.scalar.activation(hg, h_ps, AF.Gelu)
                # mean/var via bn_stats (3 subgroups of 256)
                st = stats.tile([128, 3, 6], f32, tag="bnst")
                for g in range(3):
                    nc.vector.bn_stats(out=st[:, g, :], in_=hg[:, g * 256:(g + 1) * 256])
                mv = stats.tile([128, 2], f32, tag="mv")
                nc.vector.bn_aggr(out=mv, in_=st)
                # rstd = 1/sqrt(var+eps), one Newton step for the Sqrt LUT
                sd = stats.tile([128, 1], f32, tag="sd")
                nc.scalar.activation(sd, mv[:, 1:2], AF.Sqrt, bias=eps_sb)
                r0 = stats.tile([128, 1], f32, tag="r0")
                nc.vector.reciprocal(r0, sd)
                vpe = stats.tile([128, 1], f32, tag="vpe")
                nc.vector.tensor_scalar_add(vpe, mv[:, 1:2], LN_EPS)
                t1 = stats.tile([128, 1], f32, tag="t1")
                nc.vector.tensor_mul(t1, r0, r0)
                nc.vector.tensor_mul(t1, t1, vpe)
                nc.vector.tensor_scalar(t1, t1, -0.5, 1.5, ALU.mult, ALU.add)
                rstd = stats.tile([128, 1], f32, tag="rstd")
                nc.vector.tensor_mul(rstd, t1, r0)
                nmu = stats.tile([128, 1], f32, tag="nmu")
                nc.vector.tensor_mul(nmu, mv[:, 0:1], rstd)
                nc.vector.tensor_scalar_mul(nmu, nmu, -1.0)
                # hn = (hg - mu) * rstd, rounded to f32r
                hn = work.tile([128, H], f32r, tag="hn")
                nc.scalar.activation(hn, hg, AF.Copy, bias=nmu, scale=rstd)
                # transpose into hnT [k, t]
                for k in range(KT):
                    tp = tpps_pool.tile([128, 128], f32r)
                    nc.tensor.transpose(tp, hn[:, k * 128:(k + 1) * 128], ident_sb)
                    nc.scalar.copy(out=hnT_sb[:, k, t * 128:(t + 1) * 128], in_=tp)

            # ---- phase 2: decode ----
            for v in range(NV):
                wd_sb = wd_pool.tile([128, KT, VTILE], f32r, tag="wd")
                nc.gpsimd.dma_start(
                    out=wd_sb,
                    in_=wdT_d[:, v * VTILE:(v + 1) * VTILE].rearrange(
                        "(c p) n -> p c n", p=128),
                )
                for t in range(NT):
                    ps = decps_pool.tile([128, VTILE], f32)
                    for k in range(KT):
                        nc.tensor.matmul(
                            ps,
                            hnT_sb[:, k, t * 128:(t + 1) * 128],
                            wd_sb[:, k, :],
                            start=(k == 0),
                            stop=(k == KT - 1 and not with_bd),
                        )
                    if with_bd:
                        nc.tensor.matmul(
                            ps, ones_sb, bdT_sb[0:1, v * VTILE:(v + 1) * VTILE],
                            start=False, stop=True,
                        )
                    ot = outp.tile([128, VTILE], f32, tag="ot")
                    if (v * NT + t) % 2 == 0:
                        nc.scalar.copy(out=ot, in_=ps)
                    else:
                        nc.vector.tensor_copy(out=ot, in_=ps)
                    nc.sync.dma_start(out=out_tiled[t, :, v, :], in_=ot)

    nc.compile()
    return nc


def _get_program(T_cap, V_cap, with_bt, with_bd):
    key = (T_cap, V_cap, with_bt, with_bd)
    if key not in _prog_cache:
        _prog_cache[key] = _build_program(T_cap, V_cap, with_bt, with_bd)
    return _prog_cache[key]


def kernel(last_hidden_states, cluster_labels, W_t, b_t, ln_gamma, ln_beta,
           dec_weights, dec_biases):
    from concourse import bass_utils

    x = np.ascontiguousarray(np.asarray(last_hidden_states, dtype=np.float32))
    B, S, Hx = x.shape
    assert Hx == H
    labels = np.asarray(cluster_labels).reshape(-1)
    W_t = np.asarray(W_t, dtype=np.float32)
    b_t = np.asarray(b_t, dtype=np.float32)
    gamma = np.asarray(ln_gamma, dtype=np.float32)
    beta = np.asarray(ln_beta, dtype=np.float32)

    x_flat = x.reshape(-1, H)
    idxs = [np.nonzero(labels == i)[0] for i in range(N_CLUSTERS)]
    counts = [len(ix) for ix in idxs]
    T_cap = max(128, ((max(counts) + 127) // 128) * 128)
    V_cap = max(((v + VTILE - 1) // VTILE) * VTILE for v in VOCABS)

    with_bt = bool(np.any(b_t != 0.0))
    wtT = np.ascontiguousarray(W_t.T)

    # fold LN affine into decoder weights/biases:
    # (hn*gamma+beta) @ Wd^T + b == hn @ (Wd*gamma)^T + (b + Wd@beta)
    wdTs, bds = [], []
    for i in range(N_CLUSTERS):
        wd = np.asarray(dec_weights[i], dtype=np.float32)
        bd = np.asarray(dec_biases[i], dtype=np.float32)
        v = wd.shape[0]
        wdT = np.zeros((H, V_cap), dtype=np.float32)
        np.matmul(np.diag(gamma), wd.T, out=wdT[:, :v]) if False else None
        wdT[:, :v] = (wd * gamma[None, :]).T
        bfold = np.zeros((V_cap,), dtype=np.float32)
        bfold[:v] = bd + wd @ beta
        wdTs.append(wdT)
        bds.append(bfold)
    with_bd = bool(any(np.any(b != 0.0) for b in bds))

    nc = _get_program(T_cap, V_cap, with_bt, with_bd)

    ident = np.eye(128, dtype=np.float32)
    ones1 = np.ones((1, 128), dtype=np.float32)

    in_maps = []
    for i in range(N_CLUSTERS):
        xT = np.zeros((H, T_cap), dtype=np.float32)
        if counts[i]:
            xT[:, :counts[i]] = x_flat[idxs[i]].T
        m = {"xT": xT, "wtT": wtT, "wdT": wdTs[i], "ident": ident, "ones1": ones1}
        if with_bt:
            m["btT"] = b_t.reshape(1, H)
        if with_bd:
            m["bdT"] = bds[i].reshape(1, V_cap)
        in_maps.append(m)

    res = bass_utils.run_bass_kernel_spmd(nc, in_maps, core_ids=list(range(8)))

    outs = []
    for i in range(N_CLUSTERS):
        v = VOCABS[i]
        full = np.zeros((B * S, v), dtype=np.float32)
        if counts[i]:
            full[idxs[i]] = res.results[i]["out"][:counts[i], :v]
        outs.append(full.reshape(B, S, v))
    return tuple(outs)


# revision 9
# speedup vs baseline: 1.0675x; 1.0675x over previous
"""Trainium2 Bass kernel for nn_ClusterClassifier (moe_routing).

Strategy: expert-parallel with host-side token routing.
  - Host groups tokens by cluster label; core i gets cluster i's tokens
    (gathered + transposed to [H, T]) and cluster i's decoder weight
    (pre-transposed to [H, vocab_i], LN gamma folded in, zero-padded).
  - Device (per core): h = xT.T @ W_t^T  -> erf-GELU -> LayerNorm
    (bn_stats + Newton-refined rsqrt) -> PE transpose -> logits =
    hn @ Wd'^T (+ folded bias) -> DMA out [T_cap, V_cap] fp32.
  - Host scatters the compact per-cluster logits into zero-filled full
    outputs (the mask semantics of the reference).

All matmuls run in float32r (TF32-class fp32 fast path, ~1e-4 rel err).
"""

import numpy as np

H = 768
KT = H // 128  # 6 contraction chunks
N_CLUSTERS = 8
VOCABS = [3000, 4000, 5000, 6000, 3000, 4000, 5000, 6000]
LN_EPS = 1e-12
VTILE = 512

_prog_cache: dict = {}


def _build_program(T_cap: int, V_cap: int, with_bt: bool, with_bd: bool,
                   use_gelu: bool = True, reps: int = 1):
    import concourse.bacc as bacc
    import concourse.mybir as mybir
    import concourse.tile as tile

    f32 = mybir.dt.float32
    f32r = mybir.dt.float32r
    AF = mybir.ActivationFunctionType
    ALU = mybir.AluOpType

    NT = T_cap // 128
    NV = V_cap // VTILE

    nc = bacc.Bacc("TRN2", target_bir_lowering=False, debug=False, num_devices=8)

    xT_d = nc.dram_tensor("xT", [H, T_cap], f32, kind="ExternalInput").ap()
    wtT_d = nc.dram_tensor("wtT", [H, H], f32, kind="ExternalInput").ap()
    wdT_d = nc.dram_tensor("wdT", [H, V_cap], f32, kind="ExternalInput").ap()
    id_d = nc.dram_tensor("ident", [128, 128], f32, kind="ExternalInput").ap()
    ones_d = nc.dram_tensor("ones1", [1, 128], f32, kind="ExternalInput").ap()
    if with_bt:
        bt_d = nc.dram_tensor("btT", [1, H], f32, kind="ExternalInput").ap()
    if with_bd:
        bd_d = nc.dram_tensor("bdT", [1, V_cap], f32, kind="ExternalInput").ap()
    out_d = nc.dram_tensor("out", [T_cap, V_cap], f32, kind="ExternalOutput").ap()

    out_tiled = out_d.rearrange("(nt p) (nv n) -> nt p nv n", p=128, n=VTILE)

    with tile.TileContext(nc) as tc:
        with (
            tc.tile_pool(name="consts", bufs=1) as consts,
            tc.tile_pool(name="hnt", bufs=1) as hnt_pool,
            tc.tile_pool(name="wd", bufs=3) as wd_pool,
            tc.tile_pool(name="work", bufs=2) as work,
            tc.tile_pool(name="stats", bufs=4) as stats,
            tc.tile_pool(name="outp", bufs=6) as outp,
            tc.tile_pool(name="hps", bufs=1, space="PSUM") as hps_pool,
            tc.tile_pool(name="tpps", bufs=2, space="PSUM") as tpps_pool,
            tc.tile_pool(name="decps", bufs=4, space="PSUM") as decps_pool,
        ):
            # ---- constants / full-kernel-lifetime tensors ----
            ident_sb = consts.tile([128, 128], f32r)
            nc.gpsimd.dma_start(out=ident_sb, in_=id_d)
            ones_sb = consts.tile([1, 128], f32r)
            nc.gpsimd.dma_start(out=ones_sb, in_=ones_d)
            eps_sb = consts.tile([128, 1], f32)
            nc.vector.memset(eps_sb, LN_EPS)
            if with_bt:
                btT_sb = consts.tile([1, H], f32r)
                nc.gpsimd.dma_start(out=btT_sb, in_=bt_d)
            if with_bd:
                bdT_sb = consts.tile([1, V_cap], f32r)
                nc.gpsimd.dma_start(out=bdT_sb, in_=bd_d)

            for _rep in range(reps):
                run_body(nc, tc, consts, hnt_pool, wd_pool, work, stats, outp,
                         hps_pool, tpps_pool, decps_pool, xT_d, wtT_d, wdT_d,
                         out_tiled, ident_sb, ones_sb, eps_sb,
                         btT_sb if with_bt else None,
                         bdT_sb if with_bd else None,
                         T_cap, V_cap, use_gelu)

    nc.compile()
    return nc


def run_body(nc, tc, consts, hnt_pool, wd_pool, work, stats, outp,
             hps_pool, tpps_pool, decps_pool, xT_d, wtT_d, wdT_d,
             out_tiled, ident_sb, ones_sb, eps_sb, btT_sb, bdT_sb,
             T_cap, V_cap, use_gelu):
    import concourse.mybir as mybir
    f32 = mybir.dt.float32
    f32r = mybir.dt.float32r
    AF = mybir.ActivationFunctionType
    ALU = mybir.AluOpType
    NT = T_cap // 128
    NV = V_cap // VTILE
    with_bt = btT_sb is not None
    with_bd = bdT_sb is not None

    xT_sb = consts.tile([128, KT, T_cap], f32r, tag="xT")
    nc.gpsimd.dma_start(out=xT_sb, in_=xT_d.rearrange("(c p) t -> p c t", p=128))
    wtT_sb = consts.tile([128, KT, H], f32r, tag="wtT")
    nc.gpsimd.dma_start(out=wtT_sb, in_=wtT_d.rearrange("(c p) o -> p c o", p=128))

    if True:
        if True:
            hnT_sb = hnt_pool.tile([128, KT, T_cap], f32r, tag="hnT")

            # ---- phase 1: transform + layernorm + transpose ----
            for t in range(NT):
                h_ps = hps_pool.tile([128, H], f32)
                for lo, hi in ((0, 512), (512, 768)):
                    for k in range(KT):
                        nc.tensor.matmul(
                            h_ps[:, lo:hi],
                            xT_sb[:, k, t * 128:(t + 1) * 128],
                            wtT_sb[:, k, lo:hi],
                            start=(k == 0),
                            stop=(k == KT - 1 and not with_bt),
                        )
                    if with_bt:
                        nc.tensor.matmul(
                            h_ps[:, lo:hi], ones_sb, btT_sb[0:1, lo:hi],
                            start=False, stop=True,
                        )
                # GELU (erf variant) PSUM -> SBUF
                hg = work.tile([128, H], f32, tag="hg")
                nc.scalar.activation(hg, h_ps, AF.Gelu if use_gelu else AF.Identity)
                # mean/var via bn_stats (3 subgroups of 256)
                st = stats.tile([128, 3, 6], f32, tag="bnst")
                for g in range(3):
                    nc.vector.bn_stats(out=st[:, g, :], in_=hg[:, g * 256:(g + 1) * 256])
                mv = stats.tile([128, 2], f32, tag="mv")
                nc.vector.bn_aggr(out=mv, in_=st)
                # rstd = 1/sqrt(var+eps), one Newton step for the Sqrt LUT
                sd = stats.tile([128, 1], f32, tag="sd")
                nc.scalar.activation(sd, mv[:, 1:2], AF.Sqrt, bias=eps_sb)
                r0 = stats.tile([128, 1], f32, tag="r0")
                nc.vector.reciprocal(r0, sd)
                vpe = stats.tile([128, 1], f32, tag="vpe")
                nc.vector.tensor_scalar_add(vpe, mv[:, 1:2], LN_EPS)
                t1 = stats.tile([128, 1], f32, tag="t1")
                nc.vector.tensor_mul(t1, r0, r0)
                nc.vector.tensor_mul(t1, t1, vpe)
                nc.vector.tensor_scalar(t1, t1, -0.5, 1.5, ALU.mult, ALU.add)
                rstd = stats.tile([128, 1], f32, tag="rstd")
                nc.vector.tensor_mul(rstd, t1, r0)
                nmu = stats.tile([128, 1], f32, tag="nmu")
                nc.vector.tensor_mul(nmu, mv[:, 0:1], rstd)
                nc.vector.tensor_scalar_mul(nmu, nmu, -1.0)
                # hn = (hg - mu) * rstd, rounded to f32r
                hn = work.tile([128, H], f32r, tag="hn")
                nc.scalar.activation(hn, hg, AF.Identity, bias=nmu, scale=rstd)
                # transpose into hnT [k, t]
                for k in range(KT):
                    tp = tpps_pool.tile([128, 128], f32r)
                    nc.tensor.transpose(tp, hn[:, k * 128:(k + 1) * 128], ident_sb)
                    nc.scalar.copy(out=hnT_sb[:, k, t * 128:(t + 1) * 128], in_=tp)

            # ---- phase 2: decode ----
            for v in range(NV):
                wd_sb = wd_pool.tile([128, KT, VTILE], f32r, tag="wd")
                nc.gpsimd.dma_start(
                    out=wd_sb,
                    in_=wdT_d[:, v * VTILE:(v + 1) * VTILE].rearrange(
                        "(c p) n -> p c n", p=128),
                )
                for t in range(NT):
                    ps = decps_pool.tile([128, VTILE], f32)
                    for k in range(KT):
                        nc.tensor.matmul(
                            ps,
                            hnT_sb[:, k, t * 128:(t + 1) * 128],
                            wd_sb[:, k, :],
                            start=(k == 0),
                            stop=(k == KT - 1 and not with_bd),
                        )
                    if with_bd:
                        nc.tensor.matmul(
                            ps, ones_sb, bdT_sb[0:1, v * VTILE:(v + 1) * VTILE],
                            start=False, stop=True,
                        )
                    ot = outp.tile([128, VTILE], f32, tag="ot")
                    if (v * NT + t) % 2 == 0:
                        nc.scalar.copy(out=ot, in_=ps)
                    else:
                        nc.vector.tensor_copy(out=ot, in_=ps)
                    nc.sync.dma_start(out=out_tiled[t, :, v, :], in_=ot)


def _get_program(T_cap, V_cap, with_bt, with_bd):
    key = (T_cap, V_cap, with_bt, with_bd)
    if key not in _prog_cache:
        _prog_cache[key] = _build_program(T_cap, V_cap, with_bt, with_bd)
    return _prog_cache[key]


def kernel(last_hidden_states, cluster_labels, W_t, b_t, ln_gamma, ln_beta,
           dec_weights, dec_biases):
    from concourse import bass_utils

    x = np.ascontiguousarray(np.asarray(last_hidden_states, dtype=np.float32))
    B, S, Hx = x.shape
    assert Hx == H
    labels = np.asarray(cluster_labels).reshape(-1)
    W_t = np.asarray(W_t, dtype=np.float32)
    b_t = np.asarray(b_t, dtype=np.float32)
    gamma = np.asarray(ln_gamma, dtype=np.float32)
    beta = np.asarray(ln_beta, dtype=np.float32)

    x_flat = x.reshape(-1, H)
    idxs = [np.nonzero(labels == i)[0] for i in range(N_CLUSTERS)]
    counts = [len(ix) for ix in idxs]
    T_cap = max(128, ((max(counts) + 127) // 128) * 128)
    V_cap = max(((v + VTILE - 1) // VTILE) * VTILE for v in VOCABS)

    with_bt = bool(np.any(b_t != 0.0))
    wtT = np.ascontiguousarray(W_t.T)

    # fold LN affine into decoder weights/biases:
    # (hn*gamma+beta) @ Wd^T + b == hn @ (Wd*gamma)^T + (b + Wd@beta)
    wdTs, bds = [], []
    for i in range(N_CLUSTERS):
        wd = np.asarray(dec_weights[i], dtype=np.float32)
        bd = np.asarray(dec_biases[i], dtype=np.float32)
        v = wd.shape[0]
        wdT = np.zeros((H, V_cap), dtype=np.float32)
        wdT[:, :v] = (wd * gamma[None, :]).T
        bfold = np.zeros((V_cap,), dtype=np.float32)
        bfold[:v] = bd + wd @ beta
        wdTs.append(wdT)
        bds.append(bfold)
    with_bd = bool(any(np.any(b != 0.0) for b in bds))

    nc = _get_program(T_cap, V_cap, with_bt, with_bd)

    ident = np.eye(128, dtype=np.float32)
    ones1 = np.ones((1, 128), dtype=np.float32)

    in_maps = []
    for i in range(N_CLUSTERS):
        xT = np.zeros((H, T_cap), dtype=np.float32)
        if counts[i]:
            xT[:, :counts[i]] = x_flat[idxs[i]].T
        m = {"xT": xT, "wtT": wtT, "wdT": wdTs[i], "ident": ident, "ones1": ones1}
        if with_bt:
            m["btT"] = b_t.reshape(1, H)
        if with_bd:
            m["bdT"] = bds[i].reshape(1, V_cap)
        in_maps.append(m)

    res = bass_utils.run_bass_kernel_spmd(nc, in_maps, core_ids=list(range(8)))

    outs = []
    for i in range(N_CLUSTERS):
        v = VOCABS[i]
        full = np.zeros((B * S, v), dtype=np.float32)
        if counts[i]:
            full[idxs[i]] = res.results[i]["out"][:counts[i], :v]
        outs.append(full.reshape(B, S, v))
    return tuple(outs)
